# revision 2
# baseline (speedup 1.0000x reference)
"""APPNP GNN kernel for 8 Trainium2 NeuronCores — v2 (dma_gather + PE segsum).

Design:
  - Nodes partitioned across 8 cores (12500 real / 12544 padded each).
    Node n (local) lives at h row n; SBUF slot (p=n//98, b=n%98).
  - MLP (512->256->64, ReLU) data-parallel, transposed weights on PE.
  - Norm factorization: ew = 0.9*norm[s]*norm[d].  Store g = norm*h (fp16,
    rows padded to 128 cols = 256B).  Per iteration:
        acc[d]  = sum_{e: dst=d} g[src_e]          (gather + PE segment-sum)
        g_new   = A*acc + B,  A = 0.9*norm^2,  B = 0.1*norm*h0
    Final h_K = g_K / norm feeds the classifier.
  - Gather: custom InstDMAGatherAnt, int16 indices over 4 address windows
    of h_full; edges grouped (tile of 128 dsts x window), sorted by src,
    padded to multiples of 128.  Row i lands at partition i%128, slot i//128.
  - Segment-sum: per 128-edge chunk, S[e, d] = (seg_id[e] == d) built on DVE
    via is_equal against an iota row constant; PE matmul psum += S^T @ X
    accumulates per-tile [128 dst, 64 feat].
  - AllGather (fp16, 128-wide rows) rebuilds h_full each iteration.
"""

import os
import sys

import numpy as np

for _p in ("/opt/trn_rl_repo", "/root/.axon_site/_ro/trn_rl_repo"):
    if os.path.isdir(_p) and _p not in sys.path:
        sys.path.append(_p)

import concourse.bass as bass
import concourse.bacc as bacc
import concourse.mybir as mybir
import concourse.tile as tile
from concourse.ap import AP
from concourse.bass_utils import run_bass_kernel_spmd

N_NODES = 100000
IN_F = 512
H1 = 256
H2 = 64
NCLS = 40
KITER = 10
ALPHA = 0.1

CORES = 8
OWN = 12500
OWNP = 12544          # 98 * 128
NBLK = 98
NP_ALL = CORES * OWNP  # 100352
NT = 25               # MLP strips of 512 cols
EW = 128              # h row width (fp16) = 256B
WIN = 32768
WBASES = [0, 32768, 65536, 98304]
GSIZE = 3             # tiles per gather group

FP32 = mybir.dt.float32
FP16 = mybir.dt.float16
I16 = mybir.dt.int16

LAST_RESULTS = None
_prog_cache = {}


def build_program(meta):
    """meta: dict with group structure (same for all cores):
       groups: list of lists of tile ids
       gw_nidx[g][w]: num idxs for gather (g, w)  (multiple of 128, may be 0)
       gw_idxoff[g][w]: column offset into idx tensor (int16 cols, /16)
       gw_blkoff[g][w]: first xg block of gather (g, w)
       tile_chunks[j]: list of (xg_block, seg_col) in accumulation order
       maxb: xg blocks per group buffer
       idxtot: total int16 columns; chtot: total chunks
    """
    groups = meta["groups"]
    gw_nidx = meta["gw_nidx"]
    gw_idxoff = meta["gw_idxoff"]
    gw_blkoff = meta["gw_blkoff"]
    tile_chunks = meta["tile_chunks"]
    MAXB = meta["maxb"]
    IDXTOT = meta["idxtot"]
    CHTOT = meta["chtot"]

    nc = bacc.Bacc(None, target_bir_lowering=False, debug=False)

    # ---- I/O ----
    ft = nc.dram_tensor("ft", [IN_F, OWNP], FP32, kind="ExternalInput")
    w0 = nc.dram_tensor("w0", [IN_F, H1], FP32, kind="ExternalInput")
    w1 = nc.dram_tensor("w1", [H1, H2], FP32, kind="ExternalInput")
    w2 = nc.dram_tensor("w2", [H2, NCLS], FP32, kind="ExternalInput")
    b0 = nc.dram_tensor("b0", [128, 2], FP32, kind="ExternalInput")
    b1 = nc.dram_tensor("b1", [H2, 1], FP32, kind="ExternalInput")
    b2 = nc.dram_tensor("b2", [NCLS, 1], FP32, kind="ExternalInput")
    ident_in = nc.dram_tensor("ident", [128, 128], FP32, kind="ExternalInput")
    iota_in = nc.dram_tensor("iota", [128, 128], FP16, kind="ExternalInput")
    aconst_in = nc.dram_tensor("aconst", [128, NBLK], FP32,
                               kind="ExternalInput")
    n01_in = nc.dram_tensor("n01", [128, NBLK], FP32, kind="ExternalInput")
    inorm_in = nc.dram_tensor("inorm", [128, NBLK], FP32,
                              kind="ExternalInput")
    idx_in = nc.dram_tensor("idx", [128, IDXTOT], I16, kind="ExternalInput")
    seg_in = nc.dram_tensor("seg", [128, CHTOT], FP16, kind="ExternalInput")
    outT = nc.dram_tensor("outT", [NCLS, OWNP], FP32, kind="ExternalOutput")

    h_own = nc.dram_tensor("hown", [OWNP, EW], FP16)
    h_full = nc.dram_tensor("hfull", [NP_ALL, EW], FP16, addr_space="Shared")

    def own_view():
        # h_own row (p*98 + b) <-> [p, b, :]
        return h_own.ap().rearrange("(p b) f -> p b f", b=NBLK)

    with tile.TileContext(nc) as tc:
        with tc.tile_pool(name="const", bufs=1) as cpool:
            # ---- persistent constants ----
            w0_sb = cpool.tile([128, 4, H1], FP32)
            for k in range(4):
                nc.sync.dma_start(w0_sb[:, k, :], w0[k * 128:(k + 1) * 128, :])
            w1_sb = cpool.tile([128, 2, H2], FP32)
            for k in range(2):
                nc.sync.dma_start(w1_sb[:, k, :], w1[k * 128:(k + 1) * 128, :])
            w2_sb = cpool.tile([H2, NCLS], FP32)
            nc.sync.dma_start(w2_sb[:], w2[:, :])
            b0_sb = cpool.tile([128, 2], FP32)
            nc.sync.dma_start(b0_sb[:], b0[:, :])
            b1_sb = cpool.tile([H2, 1], FP32)
            nc.sync.dma_start(b1_sb[:], b1[:, :])
            b2_sb = cpool.tile([NCLS, 1], FP32)
            nc.sync.dma_start(b2_sb[:], b2[:, :])
            ident = cpool.tile([128, 128], FP32)
            nc.sync.dma_start(ident[:], ident_in[:, :])
            iota_sb = cpool.tile([128, 128], FP16)
            nc.sync.dma_start(iota_sb[:], iota_in[:, :])
            a_sb = cpool.tile([128, NBLK], FP32)
            nc.sync.dma_start(a_sb[:], aconst_in[:, :])
            n01_sb = cpool.tile([128, NBLK], FP32)
            nc.sync.dma_start(n01_sb[:], n01_in[:, :])
            inorm_sb = cpool.tile([128, NBLK], FP32)
            nc.sync.dma_start(inorm_sb[:], inorm_in[:, :])
            seg_sb = cpool.tile([128, CHTOT], FP16)
            nc.sync.dma_start(seg_sb[:], seg_in[:, :])

            B_sb = cpool.tile([128, NBLK, H2], FP32)      # 0.1*norm*h0
            g_sb = cpool.tile([128, NBLK, EW], FP16)      # g, padded cols

            # zero g_sb (incl. padding cols) once
            nc.vector.memset(g_sb[:], 0.0)

            # ---- Phase 1: MLP ----
            with (
                tc.tile_pool(name="mlp", bufs=2) as mpool,
                tc.tile_pool(name="mlppsum", bufs=2, space="PSUM") as mpsum,
            ):
                for j in range(NT):
                    nt = min(512, OWNP - j * 512)
                    nb = nt // 128
                    ft_sb = mpool.tile([128, 4, 512], FP32, tag="ft")
                    for k in range(4):
                        nc.sync.dma_start(
                            ft_sb[:, k, :nt],
                            ft[k * 128:(k + 1) * 128, j * 512:j * 512 + nt],
                        )
                    ps0 = mpsum.tile([128, 2, 512], FP32, tag="ps0")
                    for m in range(2):
                        for k in range(4):
                            nc.tensor.matmul(
                                ps0[:, m, :nt],
                                w0_sb[:, k, m * 128:(m + 1) * 128],
                                ft_sb[:, k, :nt],
                                start=(k == 0),
                                stop=(k == 3),
                            )
                    hT0 = mpool.tile([128, 2, 512], FP32, tag="hT0")
                    for m in range(2):
                        nc.scalar.activation(
                            hT0[:, m, :nt], ps0[:, m, :nt],
                            mybir.ActivationFunctionType.Relu,
                            bias=b0_sb[:, m:m + 1],
                        )
                    ps1 = mpsum.tile([H2, 512], FP32, tag="ps1")
                    for k in range(2):
                        nc.tensor.matmul(
                            ps1[:, :nt], w1_sb[:, k, :], hT0[:, k, :nt],
                            start=(k == 0), stop=(k == 1),
                        )
                    hT1 = mpool.tile([H2, 512], FP32, tag="hT1")
                    nc.scalar.activation(
                        hT1[:, :nt], ps1[:, :nt],
                        mybir.ActivationFunctionType.Relu,
                        bias=b1_sb[:, 0:1],
                    )
                    # transpose [64, 128] -> [128, 64]; slot (b=j*4+q, p)
                    for q in range(nb):
                        tp = mpsum.tile([128, H2], FP32, tag="tp")
                        nc.tensor.transpose(
                            tp[:, :], hT1[:, q * 128:(q + 1) * 128],
                            ident[:H2, :H2],
                        )
                        # B = h0 * 0.1*norm  (written per block)
                        b = j * 4 + q
                        n01col = n01_sb[:, b:b + 1]
                        n01b = AP(n01col.tensor, n01col.offset,
                                  [list(n01col.ap[0]), [0, H2]])
                        nc.vector.tensor_tensor(
                            out=B_sb[:, b, :], in0=tp[:, :], in1=n01b,
                            op=mybir.AluOpType.mult)
                        # g0 = norm*h0 = 10*B (fp16, into padded row)
                        nc.vector.tensor_scalar(
                            g_sb[:, b, 0:H2], B_sb[:, b, :], 10.0, None,
                            mybir.AluOpType.mult)

            # initial h_own write + AllGather
            nc.sync.dma_start(own_view(), g_sb[:])
            nc.gpsimd.collective_compute(
                "AllGather", mybir.AluOpType.bypass,
                replica_groups=[list(range(CORES))],
                ins=[h_own.ap().opt()], outs=[h_full.ap().opt()],
            )

            # ---- Phase 2: APPNP iterations ----
            with (
                tc.tile_pool(name="prop", bufs=2) as ppool,
                tc.tile_pool(name="spool", bufs=4) as spool,
                tc.tile_pool(name="psum", bufs=4, space="PSUM") as qpsum,
            ):
                for t in range(KITER):
                    for g, tiles in enumerate(groups):
                        xg = ppool.tile([128, MAXB, EW], FP16, tag="xg")
                        idx_sb = ppool.tile([128, meta["idxcols_max"]], I16,
                                            tag="idx")
                        nc.sync.dma_start(
                            idx_sb[:, 0:meta["g_idxcols"][g]],
                            idx_in[:, meta["g_idxoff"][g]:
                                   meta["g_idxoff"][g] + meta["g_idxcols"][g]])
                        for w in range(4):
                            n = gw_nidx[g][w]
                            if n == 0:
                                continue
                            base = WBASES[w]
                            wsz = min(WIN, NP_ALL - base)
                            blk = gw_blkoff[g][w]
                            nb = n // 128
                            io = gw_idxoff[g][w] - meta["g_idxoff"][g]
                            nc.gpsimd.dma_gather(
                                xg[:, blk:blk + nb, :],
                                h_full[base:base + wsz, :],
                                idx_sb[:, io:io + n // 16],
                                n, n, EW,
                                single_packet=(n <= 1024),
                            )
                        for j in tiles:
                            chunks = tile_chunks[j]
                            ps = qpsum.tile([128, H2], FP32, tag="ps")
                            for ci, (xb, sc) in enumerate(chunks):
                                scol = seg_sb[:, sc:sc + 1]
                                segb = AP(scol.tensor, scol.offset,
                                          [list(scol.ap[0]), [0, 128]])
                                S = spool.tile([128, 128], FP16, tag="S")
                                nc.vector.tensor_tensor(
                                    out=S[:], in0=segb, in1=iota_sb[:],
                                    op=mybir.AluOpType.is_equal)
                                nc.tensor.matmul(
                                    ps[:], S[:], xg[:, xb, 0:H2],
                                    start=(ci == 0),
                                    stop=(ci == len(chunks) - 1),
                                )
                            # g_new = A*acc + B
                            acol = a_sb[:, j:j + 1]
                            ab = AP(acol.tensor, acol.offset,
                                    [list(acol.ap[0]), [0, H2]])
                            tmp = spool.tile([128, H2], FP32, tag="tmp")
                            nc.vector.tensor_tensor(
                                out=tmp[:], in0=ps[:], in1=ab,
                                op=mybir.AluOpType.mult)
                            nc.vector.tensor_tensor(
                                out=g_sb[:, j, 0:H2], in0=tmp[:],
                                in1=B_sb[:, j, :], op=mybir.AluOpType.add)
                    if t < KITER - 1:
                        nc.sync.dma_start(own_view(), g_sb[:])
                        nc.gpsimd.collective_compute(
                            "AllGather", mybir.AluOpType.bypass,
                            replica_groups=[list(range(CORES))],
                            ins=[h_own.ap().opt()], outs=[h_full.ap().opt()],
                        )

            # ---- Phase 3: classifier ----
            with (
                tc.tile_pool(name="cls", bufs=2) as kpool,
                tc.tile_pool(name="clspsum", bufs=2, space="PSUM") as kpsum,
            ):
                for j in range(NT):
                    nt = min(512, OWNP - j * 512)
                    nb = nt // 128
                    hT = kpool.tile([H2, 512], FP32, tag="hT")
                    for q in range(nb):
                        b = j * 4 + q
                        icol = inorm_sb[:, b:b + 1]
                        ib = AP(icol.tensor, icol.offset,
                                [list(icol.ap[0]), [0, H2]])
                        hq = kpool.tile([128, H2], FP32, tag="hq")
                        nc.vector.tensor_tensor(
                            out=hq[:], in0=g_sb[:, b, 0:H2], in1=ib,
                            op=mybir.AluOpType.mult)
                        tq = kpsum.tile([H2, 128], FP32, tag="tq")
                        nc.tensor.transpose(tq[:, :], hq[:], ident[:, :])
                        nc.vector.tensor_copy(hT[:, q * 128:(q + 1) * 128],
                                              tq[:, :])
                    pc = kpsum.tile([NCLS, 512], FP32, tag="pc")
                    nc.tensor.matmul(pc[:, :nt], w2_sb[:], hT[:, :nt],
                                     start=True, stop=True)
                    ob = kpool.tile([NCLS, 512], FP32, tag="ob")
                    b2_b = AP(b2_sb[:].tensor, b2_sb[:].offset,
                              [list(b2_sb[:].ap[0]), [0, nt]])
                    nc.vector.tensor_tensor(
                        out=ob[:, :nt], in0=pc[:, :nt], in1=b2_b,
                        op=mybir.AluOpType.add,
                    )
                    nc.sync.dma_start(outT[:, j * 512:j * 512 + nt],
                                      ob[:, :nt])

    nc.compile()
    return nc


def _prep(src, dst, cid):
    """Edge structure for core cid.  Node n (global) -> gid owner*12544 + n%12500.
    Returns per-core idx/seg arrays + shared-shape metadata pieces."""
    m = (dst // OWN) == cid
    dl = dst[m] - cid * OWN                      # local dst 0..12499
    sg = src[m]
    s_owner = sg // OWN
    gid = s_owner * OWNP + (sg - s_owner * OWN)  # row in h_full

    # h row of node n is n; SBUF slot (p = n//98, b = n%98); PSUM tile = b
    tile_id = dl % NBLK
    slot = dl // NBLK
    win = gid // WIN
    loc = gid % WIN

    # sort by (tile, win, loc)
    key = (tile_id * 4 + win) * WIN + loc
    order = np.argsort(key, kind="stable")
    tile_s, win_s, loc_s, slot_s = (tile_id[order], win[order],
                                    loc[order], slot[order])

    # counts per (tile, win)
    cnt = np.zeros((NBLK, 4), np.int64)
    np.add.at(cnt, (tile_s, win_s), 1)
    cpad = ((cnt + 127) // 128) * 128            # padded counts
    return cnt, cpad, loc_s, slot_s


def _build_core_arrays(cnt, loc_s, slot_s, meta):
    """Emit idx (int16) and seg (fp16) arrays matching the SHARED meta
    layout (cpad_max-based positions)."""
    IDXTOT = meta["idxtot"]
    CHTOT = meta["chtot"]
    cpad_max = meta["cpad_max"]
    idx_arr = np.zeros((128, IDXTOT), np.int16)
    seg_arr = np.full((128, CHTOT), 128.0, np.float16)

    # per (tile, win) start offsets into the sorted edge stream
    starts = np.zeros((NBLK, 4), np.int64)
    flat = cnt.reshape(-1)
    starts.reshape(-1)[1:] = np.cumsum(flat)[:-1]

    for g, tiles in enumerate(meta["groups"]):
        for w in range(4):
            n = meta["gw_nidx"][g][w]
            if n == 0:
                continue
            io = meta["gw_idxoff"][g][w]
            # fill member tiles at cpad_max-aligned positions
            locs = np.zeros(n, np.int64)       # padded idx stream
            segs = np.full(n, 128, np.int64)
            pos = 0
            for j in tiles:
                c = int(cnt[j, w])
                cp = int(cpad_max[j, w])
                s0 = int(starts[j, w])
                locs[pos:pos + c] = loc_s[s0:s0 + c]
                segs[pos:pos + c] = slot_s[s0:s0 + c]
                pos += cp
            assert pos == n
            # idx layout: i -> (i%16, i//16), replicated x8
            a = np.zeros((16, n // 16), np.int16)
            a[np.arange(n) % 16, np.arange(n) // 16] = locs.astype(np.int16)
            idx_arr[:, io:io + n // 16] = np.tile(a, (8, 1))
            # seg columns: chunk k covers edges [k*128,(k+1)*128): value = slot
            sc0 = meta["gw_segoff"][g][w]
            nb = n // 128
            seg_arr[:, sc0:sc0 + nb] = (
                segs.reshape(nb, 128).T.astype(np.float16))
    return idx_arr, seg_arr


def _build_meta(cpads):
    """Shared (max over cores) group structure so one program serves all."""
    groups = []
    i = 0
    while i < NBLK:
        groups.append(list(range(i, min(i + GSIZE, NBLK))))
        i += GSIZE
    cpad_max = np.maximum.reduce(cpads)          # [NBLK, 4] max over cores

    gw_nidx, gw_idxoff, gw_blkoff, gw_segoff = [], [], [], []
    g_idxoff, g_idxcols = [], []
    tile_chunks = [[] for _ in range(NBLK)]
    idxoff = 0
    segoff = 0
    maxb = 0
    for g, tiles in enumerate(groups):
        nidx, ioff, boff, soff = [], [], [], []
        blk = 0
        g_idxoff.append(idxoff)
        for w in range(4):
            n = int(cpad_max[tiles, w].sum())
            nidx.append(n)
            ioff.append(idxoff)
            boff.append(blk)
            soff.append(segoff)
            # chunks for each member tile
            b = blk
            s = segoff
            for j in tiles:
                nbj = int(cpad_max[j, w]) // 128
                for k in range(nbj):
                    tile_chunks[j].append((b + k, s + k))
                b += nbj
                s += nbj
            idxoff += n // 16
            segoff += n // 128
            blk += n // 128
        gw_nidx.append(nidx)
        gw_idxoff.append(ioff)
        gw_blkoff.append(boff)
        gw_segoff.append(soff)
        g_idxcols.append(idxoff - g_idxoff[g])
        maxb = max(maxb, blk)
    return {
        "groups": groups, "gw_nidx": gw_nidx, "gw_idxoff": gw_idxoff,
        "gw_blkoff": gw_blkoff, "gw_segoff": gw_segoff,
        "g_idxoff": g_idxoff, "g_idxcols": g_idxcols,
        "tile_chunks": tile_chunks, "maxb": maxb,
        "idxtot": idxoff, "chtot": segoff,
        "idxcols_max": max(g_idxcols), "cpad_max": cpad_max,
    }


def kernel(features, src, dst, W0, b0, W1, b1, W2, b2, trace=False):
    global LAST_RESULTS
    features = np.asarray(features, np.float32)
    src = np.asarray(src).astype(np.int64)
    dst = np.asarray(dst).astype(np.int64)
    W0 = np.asarray(W0, np.float32)
    W1 = np.asarray(W1, np.float32)
    W2 = np.asarray(W2, np.float32)
    b0 = np.asarray(b0, np.float32)
    b1 = np.asarray(b1, np.float32)
    b2 = np.asarray(b2, np.float32)

    deg = np.bincount(dst, minlength=N_NODES).astype(np.float64)
    norm = np.maximum(deg, 1.0) ** -0.5          # [N]

    preps = [_prep(src, dst, c) for c in range(CORES)]
    meta = _build_meta([p[1] for p in preps])

    key = meta["idxtot"]
    if key not in _prog_cache:
        _prog_cache.clear()
        _prog_cache[key] = build_program(meta)
    nc = _prog_cache[key]

    ident = np.eye(128, dtype=np.float32)
    iota = np.broadcast_to(np.arange(128, dtype=np.float16),
                           (128, 128)).copy()
    b0_in = np.ascontiguousarray(b0.reshape(2, 128).T)
    b1_in = b1.reshape(H2, 1)
    b2_in = b2.reshape(NCLS, 1)

    cols = np.arange(OWNP)
    node_of_col = (cols % 128) * NBLK + cols // 128   # MLP col -> local node

    in_maps = []
    for c in range(CORES):
        cnt, cpad, loc_s, slot_s = preps[c]
        idx_arr, seg_arr = _build_core_arrays(cnt, loc_s, slot_s, meta)
        # features for MLP: column order such that h_own row r = node r
        fl = features[c * OWN:(c + 1) * OWN]
        ftp = np.zeros((IN_F, OWNP), np.float32)
        valid = node_of_col < OWN
        ftp[:, valid] = fl[node_of_col[valid]].T
        # per-node consts at [p, b] = node p*98+b
        nl = np.ones(OWNP, np.float64)
        nl[:OWN] = norm[c * OWN:(c + 1) * OWN]
        npb = nl.reshape(128, NBLK)
        a_c = (0.9 * npb * npb).astype(np.float32)
        n01_c = (0.1 * npb).astype(np.float32)
        inorm_c = (1.0 / npb).astype(np.float32)
        in_maps.append({
            "ft": ftp, "w0": W0, "w1": W1, "w2": W2,
            "b0": b0_in, "b1": b1_in, "b2": b2_in,
            "ident": ident, "iota": iota,
            "aconst": a_c, "n01": n01_c, "inorm": inorm_c,
            "idx": idx_arr, "seg": seg_arr,
        })

    res = run_bass_kernel_spmd(nc, in_maps, core_ids=list(range(CORES)),
                               trace=trace)
    LAST_RESULTS = res

    out = np.empty((N_NODES, NCLS), np.float32)
    nodes = np.arange(OWN)
    col_of_node = (nodes % NBLK) * 128 + nodes // NBLK
    for c in range(CORES):
        out[c * OWN:(c + 1) * OWN] = res.results[c]["outT"].T[col_of_node]
    return out


# revision 3
# speedup vs baseline: 1.9130x; 1.9130x over previous
"""APPNP GNN kernel for 8 Trainium2 NeuronCores — v2 (dma_gather + PE segsum).

Design:
  - Nodes partitioned across 8 cores (12500 real / 12544 padded each).
    Node n (local) lives at h row n; SBUF slot (p=n//98, b=n%98).
  - MLP (512->256->64, ReLU) data-parallel, transposed weights on PE.
  - Norm factorization: ew = 0.9*norm[s]*norm[d].  Store g = norm*h (fp16,
    rows padded to 128 cols = 256B).  Per iteration:
        acc[d]  = sum_{e: dst=d} g[src_e]          (gather + PE segment-sum)
        g_new   = A*acc + B,  A = 0.9*norm^2,  B = 0.1*norm*h0
    Final h_K = g_K / norm feeds the classifier.
  - Gather: custom InstDMAGatherAnt, int16 indices over 4 address windows
    of h_full; edges grouped (tile of 128 dsts x window), sorted by src,
    padded to multiples of 128.  Row i lands at partition i%128, slot i//128.
  - Segment-sum: per 128-edge chunk, S[e, d] = (seg_id[e] == d) built on DVE
    via is_equal against an iota row constant; PE matmul psum += S^T @ X
    accumulates per-tile [128 dst, 64 feat].
  - AllGather (fp16, 128-wide rows) rebuilds h_full each iteration.
"""

import os
import sys

import numpy as np

for _p in ("/opt/trn_rl_repo", "/root/.axon_site/_ro/trn_rl_repo"):
    if os.path.isdir(_p) and _p not in sys.path:
        sys.path.append(_p)

import concourse.bass as bass
import concourse.bacc as bacc
import concourse.mybir as mybir
import concourse.tile as tile
from concourse.ap import AP
from concourse.bass_utils import run_bass_kernel_spmd

N_NODES = 100000
IN_F = 512
H1 = 256
H2 = 64
NCLS = 40
KITER = 10
ALPHA = 0.1

CORES = 8
OWN = 12500
OWNP = 12544          # 98 * 128
NBLK = 98
NP_ALL = CORES * OWNP  # 100352
NT = 25               # MLP strips of 512 cols
EW = 128              # h row width (fp16) = 256B
WIN = 32768
WBASES = [0, 32768, 65536, 98304]
GSIZE = 3             # tiles per gather group

FP32 = mybir.dt.float32
FP16 = mybir.dt.float16
I16 = mybir.dt.int16

LAST_RESULTS = None
_prog_cache = {}


def build_program(meta):
    """meta: dict with group structure (same for all cores):
       groups: list of lists of tile ids
       gw_nidx[g][w]: num idxs for gather (g, w)  (multiple of 128, may be 0)
       gw_idxoff[g][w]: column offset into idx tensor (int16 cols, /16)
       gw_blkoff[g][w]: first xg block of gather (g, w)
       tile_chunks[j]: list of (xg_block, seg_col) in accumulation order
       maxb: xg blocks per group buffer
       idxtot: total int16 columns; chtot: total chunks
    """
    groups = meta["groups"]
    gw_nidx = meta["gw_nidx"]
    gw_idxoff = meta["gw_idxoff"]
    gw_blkoff = meta["gw_blkoff"]
    tile_chunks = meta["tile_chunks"]
    MAXB = meta["maxb"]
    IDXTOT = meta["idxtot"]
    CHTOT = meta["chtot"]

    nc = bacc.Bacc(None, target_bir_lowering=False, debug=False,
                   num_swdge_queues=4)

    # ---- I/O ----
    ft = nc.dram_tensor("ft", [IN_F, OWNP], FP32, kind="ExternalInput")
    w0 = nc.dram_tensor("w0", [IN_F, H1], FP32, kind="ExternalInput")
    w1 = nc.dram_tensor("w1", [H1, H2], FP32, kind="ExternalInput")
    w2 = nc.dram_tensor("w2", [H2, NCLS], FP32, kind="ExternalInput")
    b0 = nc.dram_tensor("b0", [128, 2], FP32, kind="ExternalInput")
    b1 = nc.dram_tensor("b1", [H2, 1], FP32, kind="ExternalInput")
    b2 = nc.dram_tensor("b2", [NCLS, 1], FP32, kind="ExternalInput")
    ident_in = nc.dram_tensor("ident", [128, 128], FP32, kind="ExternalInput")
    iota_in = nc.dram_tensor("iota", [128, 128], FP16, kind="ExternalInput")
    aconst_in = nc.dram_tensor("aconst", [128, NBLK], FP32,
                               kind="ExternalInput")
    n01_in = nc.dram_tensor("n01", [128, NBLK], FP32, kind="ExternalInput")
    inorm_in = nc.dram_tensor("inorm", [128, NBLK], FP32,
                              kind="ExternalInput")
    idx_in = nc.dram_tensor("idx", [128, IDXTOT], I16, kind="ExternalInput")
    seg_in = nc.dram_tensor("seg", [128, CHTOT], FP16, kind="ExternalInput")
    outT = nc.dram_tensor("outT", [NCLS, OWNP], FP32, kind="ExternalOutput")

    h_own = nc.dram_tensor("hown", [OWNP, EW], FP16)
    h_full = nc.dram_tensor("hfull", [NP_ALL, EW], FP16, addr_space="Shared")

    def own_view():
        # h_own row (p*98 + b) <-> [p, b, :]
        return h_own.ap().rearrange("(p b) f -> p b f", b=NBLK)

    with tile.TileContext(nc) as tc:
        with tc.tile_pool(name="const", bufs=1) as cpool:
            # ---- persistent constants ----
            w0_sb = cpool.tile([128, 4, H1], FP32)
            for k in range(4):
                nc.sync.dma_start(w0_sb[:, k, :], w0[k * 128:(k + 1) * 128, :])
            w1_sb = cpool.tile([128, 2, H2], FP32)
            for k in range(2):
                nc.sync.dma_start(w1_sb[:, k, :], w1[k * 128:(k + 1) * 128, :])
            w2_sb = cpool.tile([H2, NCLS], FP32)
            nc.sync.dma_start(w2_sb[:], w2[:, :])
            b0_sb = cpool.tile([128, 2], FP32)
            nc.sync.dma_start(b0_sb[:], b0[:, :])
            b1_sb = cpool.tile([H2, 1], FP32)
            nc.sync.dma_start(b1_sb[:], b1[:, :])
            b2_sb = cpool.tile([NCLS, 1], FP32)
            nc.sync.dma_start(b2_sb[:], b2[:, :])
            ident = cpool.tile([128, 128], FP32)
            nc.sync.dma_start(ident[:], ident_in[:, :])
            iota_sb = cpool.tile([128, 128], FP16)
            nc.sync.dma_start(iota_sb[:], iota_in[:, :])
            a_sb = cpool.tile([128, NBLK], FP32)
            nc.sync.dma_start(a_sb[:], aconst_in[:, :])
            n01_sb = cpool.tile([128, NBLK], FP32)
            nc.sync.dma_start(n01_sb[:], n01_in[:, :])
            inorm_sb = cpool.tile([128, NBLK], FP32)
            nc.sync.dma_start(inorm_sb[:], inorm_in[:, :])
            seg_sb = cpool.tile([128, CHTOT], FP16)
            nc.sync.dma_start(seg_sb[:], seg_in[:, :])

            B_sb = cpool.tile([128, NBLK, H2], FP32)      # 0.1*norm*h0
            g_sb = cpool.tile([128, NBLK, EW], FP16)      # g, padded cols

            # zero g_sb (incl. padding cols) once
            nc.vector.memset(g_sb[:], 0.0)

            # ---- Phase 1: MLP ----
            with (
                tc.tile_pool(name="mlp", bufs=2) as mpool,
                tc.tile_pool(name="mlppsum", bufs=2, space="PSUM") as mpsum,
            ):
                for j in range(NT):
                    nt = min(512, OWNP - j * 512)
                    nb = nt // 128
                    ft_sb = mpool.tile([128, 4, 512], FP32, tag="ft")
                    for k in range(4):
                        nc.sync.dma_start(
                            ft_sb[:, k, :nt],
                            ft[k * 128:(k + 1) * 128, j * 512:j * 512 + nt],
                        )
                    ps0 = mpsum.tile([128, 2, 512], FP32, tag="ps0")
                    for m in range(2):
                        for k in range(4):
                            nc.tensor.matmul(
                                ps0[:, m, :nt],
                                w0_sb[:, k, m * 128:(m + 1) * 128],
                                ft_sb[:, k, :nt],
                                start=(k == 0),
                                stop=(k == 3),
                            )
                    hT0 = mpool.tile([128, 2, 512], FP32, tag="hT0")
                    for m in range(2):
                        nc.scalar.activation(
                            hT0[:, m, :nt], ps0[:, m, :nt],
                            mybir.ActivationFunctionType.Relu,
                            bias=b0_sb[:, m:m + 1],
                        )
                    ps1 = mpsum.tile([H2, 512], FP32, tag="ps1")
                    for k in range(2):
                        nc.tensor.matmul(
                            ps1[:, :nt], w1_sb[:, k, :], hT0[:, k, :nt],
                            start=(k == 0), stop=(k == 1),
                        )
                    hT1 = mpool.tile([H2, 512], FP32, tag="hT1")
                    nc.scalar.activation(
                        hT1[:, :nt], ps1[:, :nt],
                        mybir.ActivationFunctionType.Relu,
                        bias=b1_sb[:, 0:1],
                    )
                    # transpose [64, 128] -> [128, 64]; slot (b=j*4+q, p)
                    for q in range(nb):
                        tp = mpsum.tile([128, H2], FP32, tag="tp")
                        nc.tensor.transpose(
                            tp[:, :], hT1[:, q * 128:(q + 1) * 128],
                            ident[:H2, :H2],
                        )
                        # B = h0 * 0.1*norm  (written per block)
                        b = j * 4 + q
                        n01col = n01_sb[:, b:b + 1]
                        n01b = AP(n01col.tensor, n01col.offset,
                                  [list(n01col.ap[0]), [0, H2]])
                        nc.vector.tensor_tensor(
                            out=B_sb[:, b, :], in0=tp[:, :], in1=n01b,
                            op=mybir.AluOpType.mult)
                        # g0 = norm*h0 = 10*B (fp16, into padded row)
                        nc.vector.tensor_scalar(
                            g_sb[:, b, 0:H2], B_sb[:, b, :], 10.0, None,
                            mybir.AluOpType.mult)

            # initial h_own write + AllGather
            nc.sync.dma_start(own_view(), g_sb[:])
            nc.gpsimd.collective_compute(
                "AllGather", mybir.AluOpType.bypass,
                replica_groups=[list(range(CORES))],
                ins=[h_own.ap().opt()], outs=[h_full.ap().opt()],
            )

            # ---- Phase 2: APPNP iterations ----
            with (
                tc.tile_pool(name="prop", bufs=2) as ppool,
                tc.tile_pool(name="spool", bufs=4) as spool,
                tc.tile_pool(name="psum", bufs=4, space="PSUM") as qpsum,
            ):
                qrr = 0
                for t in range(KITER):
                    for g, tiles in enumerate(groups):
                        xg = ppool.tile([128, MAXB, EW], FP16, tag="xg")
                        idx_sb = ppool.tile([128, meta["idxcols_max"]], I16,
                                            tag="idx")
                        nc.sync.dma_start(
                            idx_sb[:, 0:meta["g_idxcols"][g]],
                            idx_in[:, meta["g_idxoff"][g]:
                                   meta["g_idxoff"][g] + meta["g_idxcols"][g]])
                        for w in range(4):
                            n = gw_nidx[g][w]
                            if n == 0:
                                continue
                            base = WBASES[w]
                            wsz = min(WIN, NP_ALL - base)
                            blk = gw_blkoff[g][w]
                            nb = n // 128
                            io = gw_idxoff[g][w] - meta["g_idxoff"][g]
                            nc.gpsimd.dma_gather(
                                xg[:, blk:blk + nb, :],
                                h_full[base:base + wsz, :],
                                idx_sb[:, io:io + n // 16],
                                n, n, EW,
                                single_packet=(n <= 1024),
                                queue_num=qrr % 4,
                            )
                            qrr += 1
                        for j in tiles:
                            runs = meta["tile_runs"][j]
                            nchunks = sum(r[2] for r in runs)
                            ps = qpsum.tile([128, H2], FP32, tag="ps")
                            ci = 0
                            for (xb0, sc0, nr) in runs:
                                segr = seg_sb[:, sc0:sc0 + nr]
                                segb = AP(segr.tensor, segr.offset,
                                          [list(segr.ap[0]),
                                           list(segr.ap[1]), [0, 128]])
                                S = spool.tile([128, meta["maxrun"], 128],
                                               FP16, tag="S")
                                nc.vector.tensor_tensor(
                                    out=S[:, 0:nr, :], in0=segb,
                                    in1=AP(iota_sb[:].tensor,
                                           iota_sb[:].offset,
                                           [list(iota_sb[:].ap[0]),
                                            [0, nr], [1, 128]]),
                                    op=mybir.AluOpType.is_equal)
                                for k in range(nr):
                                    nc.tensor.matmul(
                                        ps[:], S[:, k, :],
                                        xg[:, xb0 + k, 0:H2],
                                        start=(ci == 0),
                                        stop=(ci == nchunks - 1),
                                    )
                                    ci += 1
                            # g_new = A*acc + B
                            acol = a_sb[:, j:j + 1]
                            ab = AP(acol.tensor, acol.offset,
                                    [list(acol.ap[0]), [0, H2]])
                            tmp = spool.tile([128, H2], FP32, tag="tmp")
                            nc.vector.tensor_tensor(
                                out=tmp[:], in0=ps[:], in1=ab,
                                op=mybir.AluOpType.mult)
                            nc.vector.tensor_tensor(
                                out=g_sb[:, j, 0:H2], in0=tmp[:],
                                in1=B_sb[:, j, :], op=mybir.AluOpType.add)
                    if t < KITER - 1:
                        nc.sync.dma_start(own_view(), g_sb[:])
                        nc.gpsimd.collective_compute(
                            "AllGather", mybir.AluOpType.bypass,
                            replica_groups=[list(range(CORES))],
                            ins=[h_own.ap().opt()], outs=[h_full.ap().opt()],
                        )

            # ---- Phase 3: classifier ----
            with (
                tc.tile_pool(name="cls", bufs=2) as kpool,
                tc.tile_pool(name="clspsum", bufs=2, space="PSUM") as kpsum,
            ):
                for j in range(NT):
                    nt = min(512, OWNP - j * 512)
                    nb = nt // 128
                    hT = kpool.tile([H2, 512], FP32, tag="hT")
                    for q in range(nb):
                        b = j * 4 + q
                        icol = inorm_sb[:, b:b + 1]
                        ib = AP(icol.tensor, icol.offset,
                                [list(icol.ap[0]), [0, H2]])
                        hq = kpool.tile([128, H2], FP32, tag="hq")
                        nc.vector.tensor_tensor(
                            out=hq[:], in0=g_sb[:, b, 0:H2], in1=ib,
                            op=mybir.AluOpType.mult)
                        tq = kpsum.tile([H2, 128], FP32, tag="tq")
                        nc.tensor.transpose(tq[:, :], hq[:], ident[:, :])
                        nc.vector.tensor_copy(hT[:, q * 128:(q + 1) * 128],
                                              tq[:, :])
                    pc = kpsum.tile([NCLS, 512], FP32, tag="pc")
                    nc.tensor.matmul(pc[:, :nt], w2_sb[:], hT[:, :nt],
                                     start=True, stop=True)
                    ob = kpool.tile([NCLS, 512], FP32, tag="ob")
                    b2_b = AP(b2_sb[:].tensor, b2_sb[:].offset,
                              [list(b2_sb[:].ap[0]), [0, nt]])
                    nc.vector.tensor_tensor(
                        out=ob[:, :nt], in0=pc[:, :nt], in1=b2_b,
                        op=mybir.AluOpType.add,
                    )
                    nc.sync.dma_start(outT[:, j * 512:j * 512 + nt],
                                      ob[:, :nt])

    nc.compile()
    return nc


def _prep(src, dst, cid):
    """Edge structure for core cid.  Node n (global) -> gid owner*12544 + n%12500.
    Returns per-core idx/seg arrays + shared-shape metadata pieces."""
    m = (dst // OWN) == cid
    dl = dst[m] - cid * OWN                      # local dst 0..12499
    sg = src[m]
    s_owner = sg // OWN
    gid = s_owner * OWNP + (sg - s_owner * OWN)  # row in h_full

    # h row of node n is n; SBUF slot (p = n//98, b = n%98); PSUM tile = b
    tile_id = dl % NBLK
    slot = dl // NBLK
    win = gid // WIN
    loc = gid % WIN

    # sort by (tile, win, loc)
    key = (tile_id * 4 + win) * WIN + loc
    order = np.argsort(key, kind="stable")
    tile_s, win_s, loc_s, slot_s = (tile_id[order], win[order],
                                    loc[order], slot[order])

    # counts per (tile, win)
    cnt = np.zeros((NBLK, 4), np.int64)
    np.add.at(cnt, (tile_s, win_s), 1)
    cpad = ((cnt + 127) // 128) * 128            # padded counts
    return cnt, cpad, loc_s, slot_s


def _build_core_arrays(cnt, loc_s, slot_s, meta):
    """Emit idx (int16) and seg (fp16) arrays matching the SHARED meta
    layout (cpad_max-based positions)."""
    IDXTOT = meta["idxtot"]
    CHTOT = meta["chtot"]
    cpad_max = meta["cpad_max"]
    idx_arr = np.zeros((128, IDXTOT), np.int16)
    seg_arr = np.full((128, CHTOT), 128.0, np.float16)

    # per (tile, win) start offsets into the sorted edge stream
    starts = np.zeros((NBLK, 4), np.int64)
    flat = cnt.reshape(-1)
    starts.reshape(-1)[1:] = np.cumsum(flat)[:-1]

    for g, tiles in enumerate(meta["groups"]):
        for w in range(4):
            n = meta["gw_nidx"][g][w]
            if n == 0:
                continue
            io = meta["gw_idxoff"][g][w]
            # fill member tiles at cpad_max-aligned positions
            locs = np.zeros(n, np.int64)       # padded idx stream
            segs = np.full(n, 128, np.int64)
            pos = 0
            for j in tiles:
                c = int(cnt[j, w])
                cp = int(cpad_max[j, w])
                s0 = int(starts[j, w])
                locs[pos:pos + c] = loc_s[s0:s0 + c]
                segs[pos:pos + c] = slot_s[s0:s0 + c]
                pos += cp
            assert pos == n
            # idx layout: i -> (i%16, i//16), replicated x8
            a = np.zeros((16, n // 16), np.int16)
            a[np.arange(n) % 16, np.arange(n) // 16] = locs.astype(np.int16)
            idx_arr[:, io:io + n // 16] = np.tile(a, (8, 1))
            # seg columns: chunk k covers edges [k*128,(k+1)*128): value = slot
            sc0 = meta["gw_segoff"][g][w]
            nb = n // 128
            seg_arr[:, sc0:sc0 + nb] = (
                segs.reshape(nb, 128).T.astype(np.float16))
    return idx_arr, seg_arr


def _build_meta(cpads):
    """Shared (max over cores) group structure so one program serves all."""
    groups = []
    i = 0
    while i < NBLK:
        groups.append(list(range(i, min(i + GSIZE, NBLK))))
        i += GSIZE
    cpad_max = np.maximum.reduce(cpads)          # [NBLK, 4] max over cores

    gw_nidx, gw_idxoff, gw_blkoff, gw_segoff = [], [], [], []
    g_idxoff, g_idxcols = [], []
    tile_chunks = [[] for _ in range(NBLK)]
    tile_runs = [[] for _ in range(NBLK)]
    idxoff = 0
    segoff = 0
    maxb = 0
    maxrun = 1
    for g, tiles in enumerate(groups):
        nidx, ioff, boff, soff = [], [], [], []
        blk = 0
        g_idxoff.append(idxoff)
        for w in range(4):
            n = int(cpad_max[tiles, w].sum())
            nidx.append(n)
            ioff.append(idxoff)
            boff.append(blk)
            soff.append(segoff)
            # chunks for each member tile
            b = blk
            s = segoff
            for j in tiles:
                nbj = int(cpad_max[j, w]) // 128
                if nbj > 0:
                    tile_runs[j].append((b, s, nbj))
                    maxrun = max(maxrun, nbj)
                for k in range(nbj):
                    tile_chunks[j].append((b + k, s + k))
                b += nbj
                s += nbj
            idxoff += n // 16
            segoff += n // 128
            blk += n // 128
        gw_nidx.append(nidx)
        gw_idxoff.append(ioff)
        gw_blkoff.append(boff)
        gw_segoff.append(soff)
        g_idxcols.append(idxoff - g_idxoff[g])
        maxb = max(maxb, blk)
    return {
        "groups": groups, "gw_nidx": gw_nidx, "gw_idxoff": gw_idxoff,
        "gw_blkoff": gw_blkoff, "gw_segoff": gw_segoff,
        "g_idxoff": g_idxoff, "g_idxcols": g_idxcols,
        "tile_chunks": tile_chunks, "tile_runs": tile_runs,
        "maxb": maxb, "maxrun": maxrun,
        "idxtot": idxoff, "chtot": segoff,
        "idxcols_max": max(g_idxcols), "cpad_max": cpad_max,
    }


def kernel(features, src, dst, W0, b0, W1, b1, W2, b2, trace=False):
    global LAST_RESULTS
    features = np.asarray(features, np.float32)
    src = np.asarray(src).astype(np.int64)
    dst = np.asarray(dst).astype(np.int64)
    W0 = np.asarray(W0, np.float32)
    W1 = np.asarray(W1, np.float32)
    W2 = np.asarray(W2, np.float32)
    b0 = np.asarray(b0, np.float32)
    b1 = np.asarray(b1, np.float32)
    b2 = np.asarray(b2, np.float32)

    deg = np.bincount(dst, minlength=N_NODES).astype(np.float64)
    norm = np.maximum(deg, 1.0) ** -0.5          # [N]

    preps = [_prep(src, dst, c) for c in range(CORES)]
    meta = _build_meta([p[1] for p in preps])

    key = meta["idxtot"]
    if key not in _prog_cache:
        _prog_cache.clear()
        _prog_cache[key] = build_program(meta)
    nc = _prog_cache[key]

    ident = np.eye(128, dtype=np.float32)
    iota = np.broadcast_to(np.arange(128, dtype=np.float16),
                           (128, 128)).copy()
    b0_in = np.ascontiguousarray(b0.reshape(2, 128).T)
    b1_in = b1.reshape(H2, 1)
    b2_in = b2.reshape(NCLS, 1)

    cols = np.arange(OWNP)
    node_of_col = (cols % 128) * NBLK + cols // 128   # MLP col -> local node

    in_maps = []
    for c in range(CORES):
        cnt, cpad, loc_s, slot_s = preps[c]
        idx_arr, seg_arr = _build_core_arrays(cnt, loc_s, slot_s, meta)
        # features for MLP: column order such that h_own row r = node r
        fl = features[c * OWN:(c + 1) * OWN]
        ftp = np.zeros((IN_F, OWNP), np.float32)
        valid = node_of_col < OWN
        ftp[:, valid] = fl[node_of_col[valid]].T
        # per-node consts at [p, b] = node p*98+b
        nl = np.ones(OWNP, np.float64)
        nl[:OWN] = norm[c * OWN:(c + 1) * OWN]
        npb = nl.reshape(128, NBLK)
        a_c = (0.9 * npb * npb).astype(np.float32)
        n01_c = (0.1 * npb).astype(np.float32)
        inorm_c = (1.0 / npb).astype(np.float32)
        in_maps.append({
            "ft": ftp, "w0": W0, "w1": W1, "w2": W2,
            "b0": b0_in, "b1": b1_in, "b2": b2_in,
            "ident": ident, "iota": iota,
            "aconst": a_c, "n01": n01_c, "inorm": inorm_c,
            "idx": idx_arr, "seg": seg_arr,
        })

    res = run_bass_kernel_spmd(nc, in_maps, core_ids=list(range(CORES)),
                               trace=trace)
    LAST_RESULTS = res

    out = np.empty((N_NODES, NCLS), np.float32)
    nodes = np.arange(OWN)
    col_of_node = (nodes % NBLK) * 128 + nodes // NBLK
    for c in range(CORES):
        out[c * OWN:(c + 1) * OWN] = res.results[c]["outT"].T[col_of_node]
    return out
